# revision 54
# baseline (speedup 1.0000x reference)
"""Trainium2 Bass kernel for the CoLa MoE-routing module.

Computation (reference semantics):
    att   = q @ Wk.T + bk                  [B, S]
    a     = softmax(top8_mask(att))        [B, S]  (8 nonzero per row)
    out   = sum_s a[:, s] * (x @ V0[s].T @ V1[s].T)   [B, O]

Sharding: expert-parallel over 8 NeuronCores (8 experts each). Each core
receives the full x/q (replicated) and its slice of V0/V1. The expert axis
is rotated per-core in Wk/bk so that every core's local experts are columns
0..7 of its own attention matrix (top-k/softmax are permutation invariant).
Per-core partial outputs are summed on the host.

Shapes are hardcoded for B=256, IN=1024, OUT=1024, SUB=128, S=64, k=8.
"""

import os

import numpy as np

import concourse.bass as bass
import concourse.bacc as bacc
import concourse.mybir as mybir
import concourse.tile as tile
from concourse import bass_utils
from concourse.masks import make_identity

B = 256
IN_F = 1024
OUT_F = 1024
SUB_F = 128
Q_F = 1024
N_SUB = 64
N_ACT = 8
N_CORES = 8
E_LOC = N_SUB // N_CORES  # 8 experts per core
E_PAIRS = E_LOC // 2

P = 128
BT = B // P  # 2 batch tiles
KC = IN_F // P  # 8 contraction chunks
QC = Q_F // P
KH = KC // 2  # half of the chunks (split loads)

F32 = mybir.dt.float32
F32R = mybir.dt.float32r
BF16 = mybir.dt.bfloat16

# "fp32" (exact, slow PE), "bf16" (full-rate PE, half DMA)
MOE_DTYPE = os.environ.get("MOE_DTYPE", "bf16")
# routing-weight broadcast scheme:
#   "flat": one PE transpose per batch tile + sbuf->sbuf flatten DMA +
#           partition-0 gpsimd broadcasts (cheapest PE)
#   "tp":   per-expert column transposes + gpsimd partition_broadcast
MOE_ABC = os.environ.get("MOE_ABC", "flat")
# PE warmup matmuls to lift the HAM clock gate before real work arrives
WARMUP_MMS = int(os.environ.get("MOE_WARMUP", "9"))


def _bcast_ap(row):
    """[1, N] SBUF AP -> [128, N] partition-broadcast AP (stride-0)."""
    return bass.AP(tensor=row.tensor, offset=row.offset,
                   ap=[[0, P]] + [list(d) for d in row.ap[1:]])


def _build(mode: str):
    wdt = BF16 if mode == "bf16" else F32
    nc = bacc.Bacc("TRN2", target_bir_lowering=False, debug=False,
                   num_devices=N_CORES)

    # ---- DRAM I/O (per-core), partition-major so DMAs are contiguous ----
    qT_d = nc.dram_tensor("qT", [P, QC, B], F32, kind="ExternalInput").ap()
    # wkT carries bk in its last 64 columns (saves a tiny DMA round-trip)
    wkT_d = nc.dram_tensor("wkT", [P, (QC + 1) * N_SUB], F32,
                           kind="ExternalInput").ap()
    xT_d = nc.dram_tensor("xT", [P, KC, B], wdt, kind="ExternalInput").ap()
    v0t_d = nc.dram_tensor("v0t", [2, P, 4, KC, SUB_F], wdt,
                           kind="ExternalInput").ap()
    v1t_d = nc.dram_tensor("v1t", [2, P, 4, OUT_F], wdt,
                           kind="ExternalInput").ap()
    out_d = nc.dram_tensor("out_p", [B, OUT_F], F32, kind="ExternalOutput").ap()

    with tile.TileContext(nc) as tc:
        with (
            tc.tile_pool(name="singles", bufs=1) as singles,
            tc.tile_pool(name="weights", bufs=2) as wpool,
            tc.tile_pool(name="work", bufs=3) as work,
            tc.tile_pool(name="ps_misc", bufs=1, space="PSUM") as ps_misc,
            tc.tile_pool(name="ps_h", bufs=2, space="PSUM") as ps_h,
            tc.tile_pool(name="ps_out", bufs=1, space="PSUM") as ps_out,
        ):
            # ---- constants ----
            ones_sb = singles.tile([1, P], F32, tag="ones")
            nc.vector.memset(ones_sb, 1.0)
            ident_sb = singles.tile([P, P], F32, tag="ident")
            make_identity(nc, ident_sb)
            # keep the gpsimd queue warm so the first real broadcast
            # dispatches without a wakeup stall
            gp_warm = singles.tile([P, 8], F32, tag="gp_warm")
            nc.gpsimd.partition_broadcast(gp_warm, ident_sb[0:1, 0:8])

            # warm up the PE (HAM clock gate releases after ~3.4us of
            # sustained activity) while DMAs are still in flight
            warm_ps = ps_misc.tile([P, P], F32, tag="ps_misc")
            for _ in range(WARMUP_MMS):
                nc.tensor.matmul(warm_ps, lhsT=ident_sb, rhs=ident_sb,
                                 start=True, stop=True)

            # routing inputs split across both rings (attention gates the
            # expert loop); weights follow immediately on each ring
            wkT_sb = singles.tile([P, (QC + 1) * N_SUB], F32, tag="wkT")
            nc.sync.dma_start(wkT_sb, wkT_d)
            qT_sb = []
            for h in range(2):
                t = singles.tile([P, KH, B], F32, tag=f"qT{h}",
                                 name=f"qT{h}")
                if h == 0:
                    # split so the first chunks land earlier on the sync ring
                    nc.sync.dma_start(t[:, 0:2, :], qT_d[:, 0:2, :])
                    nc.sync.dma_start(t[:, 2:4, :], qT_d[:, 2:4, :])
                else:
                    nc.scalar.dma_start(t, qT_d[:, KH:KC, :])
                qT_sb.append(t)
            # x on the scalar ring behind qT1
            xT_sb = []
            for h in range(2):
                t = singles.tile([P, KH, B], wdt, tag=f"xT{h}",
                                 name=f"xT{h}")
                nc.scalar.dma_start(t, xT_d[:, h * KH:(h + 1) * KH, :])
                xT_sb.append(t)

            # ---- expert weights: 2 big DMAs per tensor, v0 on sync ring,
            # v1 on scalar ring (parallel streams) ----
            v0t_sb = []
            v1t_sb = []
            for m in range(2):
                t0 = wpool.tile([P, 4, KC, SUB_F], wdt, tag="v0t",
                                name=f"v0t{m}")
                nc.sync.dma_start(t0, v0t_d[m])
                v0t_sb.append(t0)
            for m in range(2):
                t1 = wpool.tile([P, 4, OUT_F], wdt, tag="v1t",
                                name=f"v1t{m}")
                nc.scalar.dma_start(t1, v1t_d[m])
                v1t_sb.append(t1)

            # ---- routing: att = q @ Wk.T + bk, per batch tile ----
            # att groups borrow the (not-yet-used) out-accumulator banks so
            # the two groups accumulate in parallel without extra PSUM
            att_ps = [ps_out.tile([P, N_SUB], F32, tag=f"out{bt}",
                                  name=f"att{bt}") for bt in range(BT)]
            aT8_sb = singles.tile([E_LOC, B], F32, tag="aT8")
            flat_sb = singles.tile([1, E_LOC * B], F32, tag="flat")
            if MOE_ABC == "flat":
                # expert 0 reads aT8 row 0 directly; the rest go through
                # the flat hop
                aT_j = {}
            else:
                aT_j = {j: singles.tile([1, B], F32, tag=f"aT{j}",
                                        name=f"aT{j}")
                        for j in range(E_LOC)}
            # all attention matmuls first (both batch tiles) so both groups
            # close as soon as qT lands; chunks ordered by DMA arrival
            order = [4, 5, 6, 7, 0, 1, 2, 3]
            for i, c in enumerate(order):
                for bt in range(BT):
                    nc.tensor.matmul(
                        att_ps[bt],
                        lhsT=qT_sb[c // KH][:, c % KH, bt * P:(bt + 1) * P],
                        rhs=wkT_sb[:, c * N_SUB:(c + 1) * N_SUB],
                        start=(i == 0), stop=False,
                    )
            for bt in range(BT):
                # bias: att += 1 (x) bk   (K=1 matmul)
                nc.tensor.matmul(
                    att_ps[bt], lhsT=ones_sb,
                    rhs=wkT_sb[0:1, QC * N_SUB:(QC + 1) * N_SUB],
                    start=False, stop=True)

            routing_hp = tc.high_priority()
            routing_hp.__enter__()
            for bt in range(BT):
                # ---- top-8 + softmax (rows = batch) ----
                # |att| <= ~5 so exp(att) cannot overflow: skip the max
                # shift; max8 (for the top-8 threshold) runs in parallel
                m8 = work.tile([P, 8], F32, tag="m8")
                nc.vector.max(out=m8, in_=att_ps[bt])
                e_top = work.tile([P, N_SUB], F32, tag="e_top")
                nc.scalar.activation(e_top, att_ps[bt],
                                     mybir.ActivationFunctionType.Exp)
                # e = (att >= t8) * e_top, denom = row-sum(e), in one op
                e = work.tile([P, N_SUB], F32, tag="e")
                denom = work.tile([P, 1], F32, tag="denom")
                nc.vector.scalar_tensor_tensor(
                    e, att_ps[bt], m8[:, 7:8], e_top,
                    op0=mybir.AluOpType.is_ge, op1=mybir.AluOpType.mult,
                    accum_out=denom)
                recip = work.tile([P, 1], F32, tag="recip")
                nc.vector.reciprocal(recip, denom)
                a_sb = work.tile([P, N_SUB], F32, tag="a_sb")
                nc.vector.tensor_scalar_mul(a_sb, e, recip)

                if MOE_ABC == "flat":
                    # one transpose of the local-expert block -> [8, P]
                    aT8_ps = ps_misc.tile([E_LOC, P], F32, tag="ps_misc")
                    nc.tensor.transpose(aT8_ps, a_sb[:, 0:E_LOC], ident_sb)
                    nc.vector.tensor_copy(aT8_sb[:, bt * P:(bt + 1) * P],
                                          aT8_ps)
                # dedicated partition-0 rows for the early experts
                for j in sorted(aT_j):
                    aTj_ps = ps_misc.tile([1, P], F32, tag="ps_misc")
                    nc.tensor.transpose(aTj_ps, a_sb[:, j:j + 1],
                                        ident_sb)
                    nc.vector.tensor_copy(
                        aT_j[j][:, bt * P:(bt + 1) * P], aTj_ps)

            if MOE_ABC == "flat":
                # flatten the 8 expert rows into one partition so
                # partition_broadcast can address each (sbuf->sbuf DMA)
                nc.sync.dma_start(flat_sb, aT8_sb)
            routing_hp.__exit__(None, None, None)

            # ---- expert loop ----
            out_ps = [ps_out.tile([P, OUT_F], F32, tag=f"out{bt}",
                                  name=f"out_ps{bt}")
                      for bt in range(BT)]
            for j in range(E_LOC):
                h_ps = ps_h.tile([P, B], F32, tag="h")
                for c in range(KC):
                    lhsT = v0t_sb[j // 4][:, j % 4, c, :]
                    nc.tensor.matmul(h_ps, lhsT=lhsT,
                                     rhs=xT_sb[c // KH][:, c % KH, :],
                                     start=(c == 0), stop=(c == KC - 1))
                # broadcast a[:, expert j] across partitions
                abc_sb = work.tile([P, B], F32, tag="abc")
                if MOE_ABC != "flat" or j in aT_j or j == 0:
                    src = aT8_sb[0:1, :] if (MOE_ABC == "flat" and j == 0) \
                        else aT_j[j]
                    nc.gpsimd.partition_broadcast(abc_sb, src)
                else:
                    nc.gpsimd.partition_broadcast(
                        abc_sb, flat_sb[:, j * B:(j + 1) * B])
                # hs = h * a  (PSUM x SBUF -> SBUF, cast to weight dtype)
                hs_sb = work.tile([P, B], wdt, tag="hs")
                nc.vector.tensor_tensor(hs_sb, h_ps, abc_sb,
                                        mybir.AluOpType.mult)
                for bt in range(BT):
                    for nh in range(2):
                        nc.tensor.matmul(
                            out_ps[bt][:, nh * 512:(nh + 1) * 512],
                            lhsT=hs_sb[:, bt * P:(bt + 1) * P],
                            rhs=v1t_sb[j // 4][:, j % 4,
                                               nh * 512:(nh + 1) * 512],
                            start=(j == 0), stop=(j == E_LOC - 1),
                        )

            # ---- write out (per 512-col region; copies split across
            # vector+scalar, DMAs on the idle sync ring) ----
            for bt in range(BT):
                for nh in range(2):
                    o_sb = work.tile([P, 512], F32, tag="o_sb")
                    src = out_ps[bt][:, nh * 512:(nh + 1) * 512]
                    if nh == 0:
                        nc.vector.tensor_copy(o_sb, src)
                    else:
                        nc.scalar.activation(
                            o_sb, src, mybir.ActivationFunctionType.Copy)
                    eng = nc.sync if bt == 0 else nc.scalar
                    eng.dma_start(
                        out_d[bt * P:(bt + 1) * P, nh * 512:(nh + 1) * 512],
                        o_sb)

    nc.compile()
    return nc


_CACHE = {}


def _get_nc(mode: str):
    if mode not in _CACHE:
        _CACHE[mode] = _build(mode)
    return _CACHE[mode]


def _pmajor(aT):
    """[D, N] (D = C*128, row-major) -> [128, C, N] partition-major."""
    d, n = aT.shape
    return np.ascontiguousarray(
        aT.reshape(d // P, P, n).transpose(1, 0, 2))


def _prep_in_maps(x, q, Wk, bk, V0, V1, mode: str):
    import ml_dtypes
    wdt = ml_dtypes.bfloat16 if mode == "bf16" else np.float32

    qT = _pmajor(q.T.astype(np.float32))                  # [128, QC, B]
    xT = _pmajor(x.T).astype(wdt)                         # [128, KC, B]
    in_maps = []
    for c in range(N_CORES):
        rot = np.roll(np.arange(N_SUB), -E_LOC * c)
        wk_pm = _pmajor(Wk[rot].T.astype(np.float32))     # [128, QC, S]
        bk_bc = np.broadcast_to(bk[rot].astype(np.float32), (P, N_SUB))
        wkT = np.ascontiguousarray(
            np.concatenate([wk_pm.reshape(P, QC * N_SUB), bk_bc], axis=1))
        base = E_LOC * c
        # v0t group m: [128, 4, KC, SUB_F]; v1t group m: [128, 4, OUT_F]
        v0 = np.stack([_pmajor(V0[base + j].T)
                       for j in range(E_LOC)])            # [8, 128, KC, SUB]
        v0t = np.ascontiguousarray(
            v0.reshape(2, 4, P, KC, SUB_F).transpose(0, 2, 1, 3, 4)
        ).astype(wdt)
        v1 = V1[base:base + E_LOC].transpose(0, 2, 1)     # [8, SUB, OUT]
        v1t = np.ascontiguousarray(
            v1.reshape(2, 4, SUB_F, OUT_F).transpose(0, 2, 1, 3)
        ).astype(wdt)
        in_maps.append({
            "qT": qT, "wkT": wkT, "xT": xT,
            "v0t": v0t, "v1t": v1t,
        })
    return in_maps


def run(inputs: dict, mode: str = MOE_DTYPE, trace: bool = False):
    """Run the distributed kernel; returns (out [B, OUT_F] fp32, results)."""
    nc = _get_nc(mode)
    in_maps = _prep_in_maps(**inputs, mode=mode)
    res = bass_utils.run_bass_kernel_spmd(
        nc, in_maps, core_ids=list(range(N_CORES)), trace=trace,
    )
    out = np.zeros((B, OUT_F), np.float32)
    for c in range(N_CORES):
        out += res.results[c]["out_p"]
    return out, res


def kernel(x, q, Wk, bk, V0, V1):
    x = np.asarray(x, np.float32)
    q = np.asarray(q, np.float32)
    Wk = np.asarray(Wk, np.float32)
    bk = np.asarray(bk, np.float32)
    V0 = np.asarray(V0, np.float32)
    V1 = np.asarray(V1, np.float32)
    out, _ = run(dict(x=x, q=q, Wk=Wk, bk=bk, V0=V0, V1=V1))
    return out


# revision 55
# speedup vs baseline: 1.0889x; 1.0889x over previous
"""Trainium2 Bass kernel for the CoLa MoE-routing module.

Computation (reference semantics):
    att   = q @ Wk.T + bk                  [B, S]
    a     = softmax(top8_mask(att))        [B, S]  (8 nonzero per row)
    out   = sum_s a[:, s] * (x @ V0[s].T @ V1[s].T)   [B, O]

Sharding: expert-parallel over 8 NeuronCores (8 experts each). Each core
receives the full x/q (replicated) and its slice of V0/V1. The expert axis
is rotated per-core in Wk/bk so that every core's local experts are columns
0..7 of its own attention matrix (top-k/softmax are permutation invariant).
Per-core partial outputs are summed on the host.

Shapes are hardcoded for B=256, IN=1024, OUT=1024, SUB=128, S=64, k=8.
"""

import os

import numpy as np

import concourse.bass as bass
import concourse.bacc as bacc
import concourse.mybir as mybir
import concourse.tile as tile
from concourse import bass_utils
from concourse.masks import make_identity

B = 256
IN_F = 1024
OUT_F = 1024
SUB_F = 128
Q_F = 1024
N_SUB = 64
N_ACT = 8
N_CORES = 8
E_LOC = N_SUB // N_CORES  # 8 experts per core
E_PAIRS = E_LOC // 2

P = 128
BT = B // P  # 2 batch tiles
KC = IN_F // P  # 8 contraction chunks
QC = Q_F // P
KH = KC // 2  # half of the chunks (split loads)

F32 = mybir.dt.float32
F32R = mybir.dt.float32r
BF16 = mybir.dt.bfloat16
FP16 = mybir.dt.float16

# "fp32" (exact, slow PE), "bf16"/"fp16" (full-rate PE, half DMA;
# fp16 has 3 more mantissa bits -> ~8x better accuracy, same speed)
MOE_DTYPE = os.environ.get("MOE_DTYPE", "fp16")
# routing-weight broadcast scheme:
#   "flat": one PE transpose per batch tile + sbuf->sbuf flatten DMA +
#           partition-0 gpsimd broadcasts (cheapest PE)
#   "tp":   per-expert column transposes + gpsimd partition_broadcast
MOE_ABC = os.environ.get("MOE_ABC", "flat")
# PE warmup matmuls to lift the HAM clock gate before real work arrives
WARMUP_MMS = int(os.environ.get("MOE_WARMUP", "9"))


def _bcast_ap(row):
    """[1, N] SBUF AP -> [128, N] partition-broadcast AP (stride-0)."""
    return bass.AP(tensor=row.tensor, offset=row.offset,
                   ap=[[0, P]] + [list(d) for d in row.ap[1:]])


def _build(mode: str):
    wdt = {"bf16": BF16, "fp16": FP16}.get(mode, F32)
    nc = bacc.Bacc("TRN2", target_bir_lowering=False, debug=False,
                   num_devices=N_CORES)

    # ---- DRAM I/O (per-core), partition-major so DMAs are contiguous ----
    qT_d = nc.dram_tensor("qT", [P, QC, B], F32, kind="ExternalInput").ap()
    # wkT carries bk in its last 64 columns (saves a tiny DMA round-trip)
    wkT_d = nc.dram_tensor("wkT", [P, (QC + 1) * N_SUB], F32,
                           kind="ExternalInput").ap()
    xT_d = nc.dram_tensor("xT", [P, KC, B], wdt, kind="ExternalInput").ap()
    v0t_d = nc.dram_tensor("v0t", [2, P, 4, KC, SUB_F], wdt,
                           kind="ExternalInput").ap()
    v1t_d = nc.dram_tensor("v1t", [2, P, 4, OUT_F], wdt,
                           kind="ExternalInput").ap()
    out_d = nc.dram_tensor("out_p", [B, OUT_F], F32, kind="ExternalOutput").ap()

    with tile.TileContext(nc) as tc:
        with (
            tc.tile_pool(name="singles", bufs=1) as singles,
            tc.tile_pool(name="weights", bufs=2) as wpool,
            tc.tile_pool(name="work", bufs=4) as work,
            tc.tile_pool(name="ps_misc", bufs=1, space="PSUM") as ps_misc,
            tc.tile_pool(name="ps_h", bufs=3, space="PSUM") as ps_h,
            tc.tile_pool(name="ps_out", bufs=1, space="PSUM") as ps_out,
        ):
            # ---- constants ----
            ones_sb = singles.tile([1, P], F32, tag="ones")
            nc.vector.memset(ones_sb, 1.0)
            ident_sb = singles.tile([P, P], F32, tag="ident")
            make_identity(nc, ident_sb)
            # keep the gpsimd queue warm so the first real broadcast
            # dispatches without a wakeup stall
            gp_warm = singles.tile([P, 8], F32, tag="gp_warm")
            nc.gpsimd.partition_broadcast(gp_warm, ident_sb[0:1, 0:8])

            # warm up the PE (HAM clock gate releases after ~3.4us of
            # sustained activity) while DMAs are still in flight
            warm_ps = ps_misc.tile([P, P], F32, tag="ps_misc")
            for _ in range(WARMUP_MMS):
                nc.tensor.matmul(warm_ps, lhsT=ident_sb, rhs=ident_sb,
                                 start=True, stop=True)

            # routing inputs split across both rings (attention gates the
            # expert loop); weights follow immediately on each ring
            wkT_sb = singles.tile([P, (QC + 1) * N_SUB], F32, tag="wkT")
            nc.sync.dma_start(wkT_sb, wkT_d)
            qT_sb = []
            for h in range(2):
                t = singles.tile([P, KH, B], F32, tag=f"qT{h}",
                                 name=f"qT{h}")
                if h == 0:
                    # split so the first chunks land earlier on the sync ring
                    nc.sync.dma_start(t[:, 0:2, :], qT_d[:, 0:2, :])
                    nc.sync.dma_start(t[:, 2:4, :], qT_d[:, 2:4, :])
                else:
                    nc.scalar.dma_start(t, qT_d[:, KH:KC, :])
                qT_sb.append(t)
            # x on the scalar ring behind qT1
            xT_sb = []
            for h in range(2):
                t = singles.tile([P, KH, B], wdt, tag=f"xT{h}",
                                 name=f"xT{h}")
                nc.scalar.dma_start(t, xT_d[:, h * KH:(h + 1) * KH, :])
                xT_sb.append(t)

            # ---- expert weights: 2 big DMAs per tensor, v0 on sync ring,
            # v1 on scalar ring (parallel streams) ----
            v0t_sb = []
            v1t_sb = []
            for m in range(2):
                t0 = wpool.tile([P, 4, KC, SUB_F], wdt, tag="v0t",
                                name=f"v0t{m}")
                nc.sync.dma_start(t0, v0t_d[m])
                v0t_sb.append(t0)
            for m in range(2):
                t1 = wpool.tile([P, 4, OUT_F], wdt, tag="v1t",
                                name=f"v1t{m}")
                nc.scalar.dma_start(t1, v1t_d[m])
                v1t_sb.append(t1)

            # ---- routing: att = q @ Wk.T + bk, per batch tile ----
            # att groups borrow the (not-yet-used) out-accumulator banks so
            # the two groups accumulate in parallel without extra PSUM
            att_ps = [ps_out.tile([P, N_SUB], F32, tag=f"out{bt}",
                                  name=f"att{bt}") for bt in range(BT)]
            aT8_sb = singles.tile([E_LOC, B], F32, tag="aT8")
            flat_sb = singles.tile([1, E_LOC * B], F32, tag="flat")
            if MOE_ABC == "flat":
                # expert 0 reads aT8 row 0 directly; the rest go through
                # the flat hop
                aT_j = {}
            else:
                aT_j = {j: singles.tile([1, B], F32, tag=f"aT{j}",
                                        name=f"aT{j}")
                        for j in range(E_LOC)}
            # all attention matmuls first (both batch tiles) so both groups
            # close as soon as qT lands; chunks ordered by DMA arrival
            order = [4, 5, 6, 7, 0, 1, 2, 3]
            for i, c in enumerate(order):
                for bt in range(BT):
                    nc.tensor.matmul(
                        att_ps[bt],
                        lhsT=qT_sb[c // KH][:, c % KH, bt * P:(bt + 1) * P],
                        rhs=wkT_sb[:, c * N_SUB:(c + 1) * N_SUB],
                        start=(i == 0), stop=False,
                    )
            for bt in range(BT):
                # bias: att += 1 (x) bk   (K=1 matmul)
                nc.tensor.matmul(
                    att_ps[bt], lhsT=ones_sb,
                    rhs=wkT_sb[0:1, QC * N_SUB:(QC + 1) * N_SUB],
                    start=False, stop=True)

            routing_hp = tc.high_priority()
            routing_hp.__enter__()
            for bt in range(BT):
                # ---- top-8 + softmax (rows = batch) ----
                # |att| <= ~5 so exp(att) cannot overflow: skip the max
                # shift; max8 (for the top-8 threshold) runs in parallel
                m8 = work.tile([P, 8], F32, tag="m8")
                nc.vector.max(out=m8, in_=att_ps[bt])
                e_top = work.tile([P, N_SUB], F32, tag="e_top")
                nc.scalar.activation(e_top, att_ps[bt],
                                     mybir.ActivationFunctionType.Exp)
                # e = (att >= t8) * e_top, denom = row-sum(e), in one op
                e = work.tile([P, N_SUB], F32, tag="e")
                denom = work.tile([P, 1], F32, tag="denom")
                nc.vector.scalar_tensor_tensor(
                    e, att_ps[bt], m8[:, 7:8], e_top,
                    op0=mybir.AluOpType.is_ge, op1=mybir.AluOpType.mult,
                    accum_out=denom)
                recip = work.tile([P, 1], F32, tag="recip")
                nc.vector.reciprocal(recip, denom)
                a_sb = work.tile([P, N_SUB], F32, tag="a_sb")
                nc.vector.tensor_scalar_mul(a_sb, e, recip)

                if MOE_ABC == "flat":
                    # one transpose of the local-expert block -> [8, P]
                    aT8_ps = ps_misc.tile([E_LOC, P], F32, tag="ps_misc")
                    nc.tensor.transpose(aT8_ps, a_sb[:, 0:E_LOC], ident_sb)
                    nc.vector.tensor_copy(aT8_sb[:, bt * P:(bt + 1) * P],
                                          aT8_ps)
                # dedicated partition-0 rows for the early experts
                for j in sorted(aT_j):
                    aTj_ps = ps_misc.tile([1, P], F32, tag="ps_misc")
                    nc.tensor.transpose(aTj_ps, a_sb[:, j:j + 1],
                                        ident_sb)
                    nc.vector.tensor_copy(
                        aT_j[j][:, bt * P:(bt + 1) * P], aTj_ps)

            if MOE_ABC == "flat":
                # flatten the 8 expert rows into one partition so
                # partition_broadcast can address each (sbuf->sbuf DMA)
                nc.sync.dma_start(flat_sb, aT8_sb)
            routing_hp.__exit__(None, None, None)

            # ---- expert loop ----
            out_ps = [ps_out.tile([P, OUT_F], F32, tag=f"out{bt}",
                                  name=f"out_ps{bt}")
                      for bt in range(BT)]
            for j in range(E_LOC):
                h_ps = ps_h.tile([P, B], F32, tag="h")
                for c in range(KC):
                    lhsT = v0t_sb[j // 4][:, j % 4, c, :]
                    nc.tensor.matmul(h_ps, lhsT=lhsT,
                                     rhs=xT_sb[c // KH][:, c % KH, :],
                                     start=(c == 0), stop=(c == KC - 1))
                # broadcast a[:, expert j] across partitions
                abc_sb = work.tile([P, B], F32, tag="abc")
                if MOE_ABC != "flat" or j in aT_j or j == 0:
                    src = aT8_sb[0:1, :] if (MOE_ABC == "flat" and j == 0) \
                        else aT_j[j]
                    nc.gpsimd.partition_broadcast(abc_sb, src)
                else:
                    nc.gpsimd.partition_broadcast(
                        abc_sb, flat_sb[:, j * B:(j + 1) * B])
                # hs = h * a  (PSUM x SBUF -> SBUF, cast to weight dtype)
                hs_sb = work.tile([P, B], wdt, tag="hs")
                nc.vector.tensor_tensor(hs_sb, h_ps, abc_sb,
                                        mybir.AluOpType.mult)
                for bt in range(BT):
                    for nh in range(2):
                        nc.tensor.matmul(
                            out_ps[bt][:, nh * 512:(nh + 1) * 512],
                            lhsT=hs_sb[:, bt * P:(bt + 1) * P],
                            rhs=v1t_sb[j // 4][:, j % 4,
                                               nh * 512:(nh + 1) * 512],
                            start=(j == 0), stop=(j == E_LOC - 1),
                        )

            # ---- write out (per 512-col region; copies split across
            # vector+scalar, DMAs on the idle sync ring) ----
            for bt in range(BT):
                for nh in range(2):
                    o_sb = work.tile([P, 512], F32, tag="o_sb")
                    src = out_ps[bt][:, nh * 512:(nh + 1) * 512]
                    if nh == 0:
                        nc.vector.tensor_copy(o_sb, src)
                    else:
                        nc.scalar.activation(
                            o_sb, src, mybir.ActivationFunctionType.Copy)
                    eng = nc.sync if bt == 0 else nc.scalar
                    eng.dma_start(
                        out_d[bt * P:(bt + 1) * P, nh * 512:(nh + 1) * 512],
                        o_sb)

    nc.compile()
    return nc


_CACHE = {}


def _get_nc(mode: str):
    if mode not in _CACHE:
        _CACHE[mode] = _build(mode)
    return _CACHE[mode]


def _pmajor(aT):
    """[D, N] (D = C*128, row-major) -> [128, C, N] partition-major."""
    d, n = aT.shape
    return np.ascontiguousarray(
        aT.reshape(d // P, P, n).transpose(1, 0, 2))


def _prep_in_maps(x, q, Wk, bk, V0, V1, mode: str):
    import ml_dtypes
    wdt = {"bf16": ml_dtypes.bfloat16, "fp16": np.float16}.get(mode, np.float32)

    qT = _pmajor(q.T.astype(np.float32))                  # [128, QC, B]
    xT = _pmajor(x.T).astype(wdt)                         # [128, KC, B]
    in_maps = []
    for c in range(N_CORES):
        rot = np.roll(np.arange(N_SUB), -E_LOC * c)
        wk_pm = _pmajor(Wk[rot].T.astype(np.float32))     # [128, QC, S]
        bk_bc = np.broadcast_to(bk[rot].astype(np.float32), (P, N_SUB))
        wkT = np.ascontiguousarray(
            np.concatenate([wk_pm.reshape(P, QC * N_SUB), bk_bc], axis=1))
        base = E_LOC * c
        # v0t group m: [128, 4, KC, SUB_F]; v1t group m: [128, 4, OUT_F]
        v0 = np.stack([_pmajor(V0[base + j].T)
                       for j in range(E_LOC)])            # [8, 128, KC, SUB]
        v0t = np.ascontiguousarray(
            v0.reshape(2, 4, P, KC, SUB_F).transpose(0, 2, 1, 3, 4)
        ).astype(wdt)
        v1 = V1[base:base + E_LOC].transpose(0, 2, 1)     # [8, SUB, OUT]
        v1t = np.ascontiguousarray(
            v1.reshape(2, 4, SUB_F, OUT_F).transpose(0, 2, 1, 3)
        ).astype(wdt)
        in_maps.append({
            "qT": qT, "wkT": wkT, "xT": xT,
            "v0t": v0t, "v1t": v1t,
        })
    return in_maps


def run(inputs: dict, mode: str = MOE_DTYPE, trace: bool = False):
    """Run the distributed kernel; returns (out [B, OUT_F] fp32, results)."""
    nc = _get_nc(mode)
    in_maps = _prep_in_maps(**inputs, mode=mode)
    res = bass_utils.run_bass_kernel_spmd(
        nc, in_maps, core_ids=list(range(N_CORES)), trace=trace,
    )
    out = np.zeros((B, OUT_F), np.float32)
    for c in range(N_CORES):
        out += res.results[c]["out_p"]
    return out, res


def kernel(x, q, Wk, bk, V0, V1):
    x = np.asarray(x, np.float32)
    q = np.asarray(q, np.float32)
    Wk = np.asarray(Wk, np.float32)
    bk = np.asarray(bk, np.float32)
    V0 = np.asarray(V0, np.float32)
    V1 = np.asarray(V1, np.float32)
    out, _ = run(dict(x=x, q=q, Wk=Wk, bk=bk, V0=V0, V1=V1))
    return out


# revision 56
# speedup vs baseline: 1.1049x; 1.0147x over previous
"""Trainium2 Bass kernel for the CoLa MoE-routing module.

Computation (reference semantics):
    att   = q @ Wk.T + bk                  [B, S]
    a     = softmax(top8_mask(att))        [B, S]  (8 nonzero per row)
    out   = sum_s a[:, s] * (x @ V0[s].T @ V1[s].T)   [B, O]

Sharding: expert-parallel over 8 NeuronCores (8 experts each). Each core
receives the full x/q (replicated) and its slice of V0/V1. The expert axis
is rotated per-core in Wk/bk so that every core's local experts are columns
0..7 of its own attention matrix (top-k/softmax are permutation invariant).
Per-core partial outputs are summed on the host.

Shapes are hardcoded for B=256, IN=1024, OUT=1024, SUB=128, S=64, k=8.
"""

import os

import numpy as np

import concourse.bass as bass
import concourse.bacc as bacc
import concourse.mybir as mybir
import concourse.tile as tile
from concourse import bass_utils
from concourse.masks import make_identity

B = 256
IN_F = 1024
OUT_F = 1024
SUB_F = 128
Q_F = 1024
N_SUB = 64
N_ACT = 8
N_CORES = 8
E_LOC = N_SUB // N_CORES  # 8 experts per core
E_PAIRS = E_LOC // 2

P = 128
BT = B // P  # 2 batch tiles
KC = IN_F // P  # 8 contraction chunks
QC = Q_F // P
KH = KC // 2  # half of the chunks (split loads)

F32 = mybir.dt.float32
F32R = mybir.dt.float32r
BF16 = mybir.dt.bfloat16
FP16 = mybir.dt.float16

# "fp32" (exact, slow PE), "bf16"/"fp16" (full-rate PE, half DMA;
# fp16 has 3 more mantissa bits -> ~8x better accuracy, same speed)
MOE_DTYPE = os.environ.get("MOE_DTYPE", "fp16")
# routing-weight broadcast scheme:
#   "flat": one PE transpose per batch tile + sbuf->sbuf flatten DMA +
#           partition-0 gpsimd broadcasts (cheapest PE)
#   "tp":   per-expert column transposes + gpsimd partition_broadcast
MOE_ABC = os.environ.get("MOE_ABC", "flat")
# PE warmup matmuls to lift the HAM clock gate before real work arrives
WARMUP_MMS = int(os.environ.get("MOE_WARMUP", "9"))


def _bcast_ap(row):
    """[1, N] SBUF AP -> [128, N] partition-broadcast AP (stride-0)."""
    return bass.AP(tensor=row.tensor, offset=row.offset,
                   ap=[[0, P]] + [list(d) for d in row.ap[1:]])


def _build(mode: str):
    wdt = {"bf16": BF16, "fp16": FP16}.get(mode, F32)
    nc = bacc.Bacc("TRN2", target_bir_lowering=False, debug=False,
                   num_devices=N_CORES)

    # ---- DRAM I/O (per-core), partition-major so DMAs are contiguous ----
    qT_d = nc.dram_tensor("qT", [P, QC, B], F32, kind="ExternalInput").ap()
    # wkT carries bk in its last 64 columns (saves a tiny DMA round-trip)
    wkT_d = nc.dram_tensor("wkT", [P, (QC + 1) * N_SUB], F32,
                           kind="ExternalInput").ap()
    xT_d = nc.dram_tensor("xT", [P, KC, B], wdt, kind="ExternalInput").ap()
    v0t_d = nc.dram_tensor("v0t", [2, P, 4, KC, SUB_F], wdt,
                           kind="ExternalInput").ap()
    v1t_d = nc.dram_tensor("v1t", [2, P, 4, OUT_F], wdt,
                           kind="ExternalInput").ap()
    out_d = nc.dram_tensor("out_p", [B, OUT_F], F32, kind="ExternalOutput").ap()

    with tile.TileContext(nc) as tc:
        with (
            tc.tile_pool(name="singles", bufs=1) as singles,
            tc.tile_pool(name="weights", bufs=2) as wpool,
            tc.tile_pool(name="work", bufs=4) as work,
            tc.tile_pool(name="ps_misc", bufs=1, space="PSUM") as ps_misc,
            tc.tile_pool(name="ps_h", bufs=3, space="PSUM") as ps_h,
            tc.tile_pool(name="ps_out", bufs=1, space="PSUM") as ps_out,
        ):
            # ---- constants ----
            ones_sb = singles.tile([1, P], F32, tag="ones")
            nc.vector.memset(ones_sb, 1.0)
            ident_sb = singles.tile([P, P], F32, tag="ident")
            make_identity(nc, ident_sb)
            # keep the gpsimd queue warm so the first real broadcast
            # dispatches without a wakeup stall
            gp_warm = singles.tile([P, 8], F32, tag="gp_warm")
            nc.gpsimd.partition_broadcast(gp_warm, ident_sb[0:1, 0:8])

            # warm up the PE (HAM clock gate releases after ~3.4us of
            # sustained activity) while DMAs are still in flight
            warm_ps = ps_misc.tile([P, P], F32, tag="ps_misc")
            for _ in range(WARMUP_MMS):
                nc.tensor.matmul(warm_ps, lhsT=ident_sb, rhs=ident_sb,
                                 start=True, stop=True)

            # routing inputs split across both rings (attention gates the
            # expert loop); weights follow immediately on each ring
            wkT_sb = singles.tile([P, (QC + 1) * N_SUB], F32, tag="wkT")
            nc.sync.dma_start(wkT_sb, wkT_d)
            qT_sb = []
            for h in range(2):
                t = singles.tile([P, KH, B], F32, tag=f"qT{h}",
                                 name=f"qT{h}")
                if h == 0:
                    # split so the first chunks land earlier on the sync ring
                    nc.sync.dma_start(t[:, 0:2, :], qT_d[:, 0:2, :])
                    nc.sync.dma_start(t[:, 2:4, :], qT_d[:, 2:4, :])
                else:
                    nc.scalar.dma_start(t, qT_d[:, KH:KC, :])
                qT_sb.append(t)
            # x on the scalar ring behind qT1
            xT_sb = []
            for h in range(2):
                t = singles.tile([P, KH, B], wdt, tag=f"xT{h}",
                                 name=f"xT{h}")
                nc.scalar.dma_start(t, xT_d[:, h * KH:(h + 1) * KH, :])
                xT_sb.append(t)

            # ---- expert weights: 2 big DMAs per tensor, v0 on sync ring,
            # v1 on scalar ring (parallel streams) ----
            v0t_sb = []
            v1t_sb = []
            for m in range(2):
                t0 = wpool.tile([P, 4, KC, SUB_F], wdt, tag="v0t",
                                name=f"v0t{m}")
                nc.sync.dma_start(t0, v0t_d[m])
                v0t_sb.append(t0)
            for m in range(2):
                t1 = wpool.tile([P, 4, OUT_F], wdt, tag="v1t",
                                name=f"v1t{m}")
                nc.scalar.dma_start(t1, v1t_d[m])
                v1t_sb.append(t1)

            # ---- routing: att = q @ Wk.T + bk, per batch tile ----
            # att groups borrow the (not-yet-used) out-accumulator banks so
            # the two groups accumulate in parallel without extra PSUM
            att_ps = [ps_out.tile([P, N_SUB], F32, tag=f"out{bt}",
                                  name=f"att{bt}") for bt in range(BT)]
            aT8_sb = singles.tile([E_LOC, B], F32, tag="aT8")
            recip_bt = [singles.tile([P, 1], F32, tag=f"recip{bt}",
                                     name=f"recip{bt}") for bt in range(BT)]
            flat_sb = singles.tile([1, E_LOC * B], F32, tag="flat")
            if MOE_ABC == "flat":
                # expert 0 reads aT8 row 0 directly; the rest go through
                # the flat hop
                aT_j = {}
            else:
                aT_j = {j: singles.tile([1, B], F32, tag=f"aT{j}",
                                        name=f"aT{j}")
                        for j in range(E_LOC)}
            # all attention matmuls first (both batch tiles) so both groups
            # close as soon as qT lands; chunks ordered by DMA arrival
            order = [4, 5, 6, 7, 0, 1, 2, 3]
            for i, c in enumerate(order):
                for bt in range(BT):
                    nc.tensor.matmul(
                        att_ps[bt],
                        lhsT=qT_sb[c // KH][:, c % KH, bt * P:(bt + 1) * P],
                        rhs=wkT_sb[:, c * N_SUB:(c + 1) * N_SUB],
                        start=(i == 0), stop=False,
                    )
            for bt in range(BT):
                # bias: att += 1 (x) bk   (K=1 matmul)
                nc.tensor.matmul(
                    att_ps[bt], lhsT=ones_sb,
                    rhs=wkT_sb[0:1, QC * N_SUB:(QC + 1) * N_SUB],
                    start=False, stop=True)

            routing_hp = tc.high_priority()
            routing_hp.__enter__()
            for bt in range(BT):
                # ---- top-8 + softmax (rows = batch) ----
                # |att| <= ~5 so exp(att) cannot overflow: skip the max
                # shift; max8 (for the top-8 threshold) runs in parallel
                m8 = work.tile([P, 8], F32, tag="m8")
                nc.vector.max(out=m8, in_=att_ps[bt])
                e_top = work.tile([P, N_SUB], F32, tag="e_top")
                nc.scalar.activation(e_top, att_ps[bt],
                                     mybir.ActivationFunctionType.Exp)
                # e = (att >= t8) * e_top, denom = row-sum(e), in one op.
                # e stays UNNORMALIZED: the 1/denom factor is applied to the
                # final output copy (out rows = batch partitions), keeping
                # the critical routing chain two hops shorter.
                e = work.tile([P, N_SUB], F32, tag="e")
                denom = work.tile([P, 1], F32, tag="denom")
                nc.vector.scalar_tensor_tensor(
                    e, att_ps[bt], m8[:, 7:8], e_top,
                    op0=mybir.AluOpType.is_ge, op1=mybir.AluOpType.mult,
                    accum_out=denom)
                nc.vector.reciprocal(recip_bt[bt], denom)
                a_sb = e

                if MOE_ABC == "flat":
                    # one transpose of the local-expert block -> [8, P]
                    aT8_ps = ps_misc.tile([E_LOC, P], F32, tag="ps_misc")
                    nc.tensor.transpose(aT8_ps, a_sb[:, 0:E_LOC], ident_sb)
                    nc.vector.tensor_copy(aT8_sb[:, bt * P:(bt + 1) * P],
                                          aT8_ps)
                # dedicated partition-0 rows for the early experts
                for j in sorted(aT_j):
                    aTj_ps = ps_misc.tile([1, P], F32, tag="ps_misc")
                    nc.tensor.transpose(aTj_ps, a_sb[:, j:j + 1],
                                        ident_sb)
                    nc.vector.tensor_copy(
                        aT_j[j][:, bt * P:(bt + 1) * P], aTj_ps)

            if MOE_ABC == "flat":
                # flatten the 8 expert rows into one partition so
                # partition_broadcast can address each (sbuf->sbuf DMA)
                nc.sync.dma_start(flat_sb, aT8_sb)
            routing_hp.__exit__(None, None, None)

            # ---- expert loop ----
            out_ps = [ps_out.tile([P, OUT_F], F32, tag=f"out{bt}",
                                  name=f"out_ps{bt}")
                      for bt in range(BT)]
            for j in range(E_LOC):
                h_ps = ps_h.tile([P, B], F32, tag="h")
                for c in range(KC):
                    lhsT = v0t_sb[j // 4][:, j % 4, c, :]
                    nc.tensor.matmul(h_ps, lhsT=lhsT,
                                     rhs=xT_sb[c // KH][:, c % KH, :],
                                     start=(c == 0), stop=(c == KC - 1))
                # broadcast a[:, expert j] across partitions
                abc_sb = work.tile([P, B], F32, tag="abc")
                if MOE_ABC != "flat" or j in aT_j or j == 0:
                    src = aT8_sb[0:1, :] if (MOE_ABC == "flat" and j == 0) \
                        else aT_j[j]
                    nc.gpsimd.partition_broadcast(abc_sb, src)
                else:
                    nc.gpsimd.partition_broadcast(
                        abc_sb, flat_sb[:, j * B:(j + 1) * B])
                # hs = h * a  (PSUM x SBUF -> SBUF, cast to weight dtype)
                hs_sb = work.tile([P, B], wdt, tag="hs")
                nc.vector.tensor_tensor(hs_sb, h_ps, abc_sb,
                                        mybir.AluOpType.mult)
                for bt in range(BT):
                    for nh in range(2):
                        nc.tensor.matmul(
                            out_ps[bt][:, nh * 512:(nh + 1) * 512],
                            lhsT=hs_sb[:, bt * P:(bt + 1) * P],
                            rhs=v1t_sb[j // 4][:, j % 4,
                                               nh * 512:(nh + 1) * 512],
                            start=(j == 0), stop=(j == E_LOC - 1),
                        )

            # ---- write out (per 512-col region; copies split across
            # vector+scalar, DMAs on the idle sync ring) ----
            for bt in range(BT):
                for nh in range(2):
                    o_sb = work.tile([P, 512], F32, tag="o_sb")
                    src = out_ps[bt][:, nh * 512:(nh + 1) * 512]
                    if nh == 0:
                        nc.vector.tensor_scalar_mul(o_sb, src, recip_bt[bt])
                    else:
                        nc.scalar.activation(
                            o_sb, src, mybir.ActivationFunctionType.Copy,
                            scale=recip_bt[bt])
                    eng = nc.sync if bt == 0 else nc.scalar
                    eng.dma_start(
                        out_d[bt * P:(bt + 1) * P, nh * 512:(nh + 1) * 512],
                        o_sb)

    nc.compile()
    return nc


_CACHE = {}


def _get_nc(mode: str):
    if mode not in _CACHE:
        _CACHE[mode] = _build(mode)
    return _CACHE[mode]


def _pmajor(aT):
    """[D, N] (D = C*128, row-major) -> [128, C, N] partition-major."""
    d, n = aT.shape
    return np.ascontiguousarray(
        aT.reshape(d // P, P, n).transpose(1, 0, 2))


def _prep_in_maps(x, q, Wk, bk, V0, V1, mode: str):
    import ml_dtypes
    wdt = {"bf16": ml_dtypes.bfloat16, "fp16": np.float16}.get(mode, np.float32)

    qT = _pmajor(q.T.astype(np.float32))                  # [128, QC, B]
    xT = _pmajor(x.T).astype(wdt)                         # [128, KC, B]
    in_maps = []
    for c in range(N_CORES):
        rot = np.roll(np.arange(N_SUB), -E_LOC * c)
        wk_pm = _pmajor(Wk[rot].T.astype(np.float32))     # [128, QC, S]
        bk_bc = np.broadcast_to(bk[rot].astype(np.float32), (P, N_SUB))
        wkT = np.ascontiguousarray(
            np.concatenate([wk_pm.reshape(P, QC * N_SUB), bk_bc], axis=1))
        base = E_LOC * c
        # v0t group m: [128, 4, KC, SUB_F]; v1t group m: [128, 4, OUT_F]
        v0 = np.stack([_pmajor(V0[base + j].T)
                       for j in range(E_LOC)])            # [8, 128, KC, SUB]
        v0t = np.ascontiguousarray(
            v0.reshape(2, 4, P, KC, SUB_F).transpose(0, 2, 1, 3, 4)
        ).astype(wdt)
        v1 = V1[base:base + E_LOC].transpose(0, 2, 1)     # [8, SUB, OUT]
        v1t = np.ascontiguousarray(
            v1.reshape(2, 4, SUB_F, OUT_F).transpose(0, 2, 1, 3)
        ).astype(wdt)
        in_maps.append({
            "qT": qT, "wkT": wkT, "xT": xT,
            "v0t": v0t, "v1t": v1t,
        })
    return in_maps


def run(inputs: dict, mode: str = MOE_DTYPE, trace: bool = False):
    """Run the distributed kernel; returns (out [B, OUT_F] fp32, results)."""
    nc = _get_nc(mode)
    in_maps = _prep_in_maps(**inputs, mode=mode)
    res = bass_utils.run_bass_kernel_spmd(
        nc, in_maps, core_ids=list(range(N_CORES)), trace=trace,
    )
    out = np.zeros((B, OUT_F), np.float32)
    for c in range(N_CORES):
        out += res.results[c]["out_p"]
    return out, res


def kernel(x, q, Wk, bk, V0, V1):
    x = np.asarray(x, np.float32)
    q = np.asarray(q, np.float32)
    Wk = np.asarray(Wk, np.float32)
    bk = np.asarray(bk, np.float32)
    V0 = np.asarray(V0, np.float32)
    V1 = np.asarray(V1, np.float32)
    out, _ = run(dict(x=x, q=q, Wk=Wk, bk=bk, V0=V0, V1=V1))
    return out
